# revision 11
# baseline (speedup 1.0000x reference)
"""Trainium2 Bass kernel for a 3-layer GRU decoder (DecoderRNN).

Math (per timestep, identical input x0 each step):
    x0 = encoder_hidden @ w_proj.T + b_proj
    3 stacked GRU layers (PyTorch gate order r,z,n), then logits = h2 @ w_out.T + b_out

Device mapping (per core, batch shard BS=4096):
  - Layout: features on SBUF partitions, batch on the free dim.
  - Weights pre-transposed/packed on host; biases ride in an extra
    contraction row against a constant 1.0 row held in each state tile.
  - Gate pre-activations accumulate in PSUM; sigmoid(r,z) is one merged
    ACT op per chunk; the n-gate bank is recycled: ghn -> (t1 read) ->
    gxn + identity@t1 -> tanh, so the gxn+r*ghn add runs on the PE.
  - w_out is packed into the layer-2 whh_n stationary; logits(t-1) ride
    the t1 multiply (rows 100:120 against a constant-ones block) straight
    into SBUF, then one DMA per step.
  - Output is [T, 24, BS] bf16 per core (rows 4:24 are logits); host
    transposes/casts back to [B, T, VOCAB] fp32.
"""

import numpy as np
import ml_dtypes

import concourse.bass as bass
import concourse.mybir as mybir
from concourse import bacc
from concourse.tile import TileContext
from concourse.bass_utils import run_bass_kernel_spmd

N_CORES = 8
B = 32768
BS = B // N_CORES  # 4096
LATENT = 128
H = 100
VOCAB = 20
VP = VOCAB + 4  # logits rows padded to a 32-aligned partition window (96:120)
T = 21
C = 512  # batch chunk = one PSUM bank of fp32

BF16 = mybir.dt.bfloat16
F32 = mybir.dt.float32
AF = mybir.ActivationFunctionType
OP = mybir.AluOpType


def build_nc(bs=BS, n_steps=T):
    nchunk = bs // C
    nhalf = bs // 2
    nc = bacc.Bacc("TRN2", target_bir_lowering=False)

    ehT = nc.declare_dram_parameter("ehT", [LATENT, bs], BF16, isOutput=False)
    wproj = nc.declare_dram_parameter("wproj", [LATENT, H], BF16, isOutput=False)
    wih = [
        nc.declare_dram_parameter(f"wih{l}", [H + 1, 3 * H], BF16, isOutput=False)
        for l in range(3)
    ]
    whh = [
        nc.declare_dram_parameter(f"whh{l}", [H + 1, 3 * H], BF16, isOutput=False)
        for l in range(3)
    ]
    # layer-2 n-gate stationary with w_out packed in cols 100:120
    wnout = nc.declare_dram_parameter("wnout", [H + 1, H + VOCAB], BF16, isOutput=False)
    ident = nc.declare_dram_parameter("ident", [H, H], BF16, isOutput=False)
    out = nc.declare_dram_parameter("out", [n_steps, VP, bs], BF16, isOutput=True)

    with TileContext(nc) as tc:
        with (
            tc.tile_pool(name="const", bufs=1) as cpool,
            tc.tile_pool(name="state", bufs=1) as spool,
            tc.tile_pool(name="rz", bufs=2) as rzpool,
            tc.tile_pool(name="work", bufs=2) as wpool,
            tc.tile_pool(name="psum", bufs=2, space="PSUM") as ppool,
            tc.tile_pool(name="psumx", bufs=1, space="PSUM") as xpool,
        ):
            # ---- load weights ----
            eh_sb = cpool.tile([LATENT, bs], BF16, tag="eh")
            nc.sync.dma_start(eh_sb[:, :], ehT[:, :])
            wproj_sb = cpool.tile([LATENT, H], BF16, tag="wproj")
            nc.sync.dma_start(wproj_sb[:, :], wproj[:, :])
            wih_sb = []
            whh_sb = []
            for l in range(3):
                wi = cpool.tile([H + 1, 3 * H], BF16, tag=f"wih{l}")
                nc.sync.dma_start(wi[:, :], wih[l][:, :])
                wih_sb.append(wi)
                wh = cpool.tile([H + 1, 3 * H], BF16, tag=f"whh{l}")
                nc.sync.dma_start(wh[:, :], whh[l][:, :])
                whh_sb.append(wh)
            wnout_sb = cpool.tile([H + 1, H + VOCAB], BF16, tag="wnout")
            nc.sync.dma_start(wnout_sb[:, :], wnout[:, :])
            ident_sb = cpool.tile([H, H], BF16, tag="ident")
            nc.sync.dma_start(ident_sb[:, :], ident[:, :])

            # ---- state tiles ----
            # ones-row writes start at partition 96 (HW requires 32-aligned
            # partition bases), then zero back rows 96:100
            h_sb = []
            for l in range(3):
                h = spool.tile([H + 1, bs], BF16, tag=f"h{l}")
                nc.gpsimd.memset(h[:, :], 0.0)
                nc.gpsimd.memset(h[96 : H + 1, :], 1.0)
                nc.gpsimd.memset(h[96:H, :], 0.0)
                h_sb.append(h)
            x0 = spool.tile([H + 1, bs], BF16, tag="x0")
            nc.gpsimd.memset(x0[96 : H + 1, :], 1.0)
            nc.gpsimd.memset(x0[96:H, :], 0.0)
            gxn0 = spool.tile([H, bs], BF16, tag="gxn0")
            # layer-2 (r|z) buffer with a constant-ones block on rows
            # 100:120 (rows 96:100 are rewritten by sigma each step)
            rz2x = spool.tile([H + VOCAB, 2 * bs], BF16, tag="rz2x")
            nc.gpsimd.memset(rz2x[96 : H + VOCAB, :], 1.0)
            # static PSUM pair tiles for the n-gate bank recycle
            xa = xpool.tile([H + VOCAB, 2 * C], F32, tag="xa")
            xb = xpool.tile([H + VOCAB, 2 * C], F32, tag="xb")

            # ---- prologue: x0 = wproj.T @ ehT ; gxn0 = wih0_n.T @ x0 ----
            for c in range(nchunk):
                sl = slice(c * C, (c + 1) * C)
                ps = ppool.tile([H, 2 * C], F32, tag="grz")
                nc.tensor.matmul(
                    ps[:, 0:C], wproj_sb[:, :], eh_sb[:, sl], start=True, stop=True
                )
                nc.scalar.copy(x0[0:H, sl], ps[:, 0:C])
            for c in range(nchunk):
                sl = slice(c * C, (c + 1) * C)
                ps = ppool.tile([H, 2 * C], F32, tag="grz")
                nc.tensor.matmul(
                    ps[:, 0:C],
                    wih_sb[0][:, 2 * H : 3 * H],
                    x0[:, sl],
                    start=True,
                    stop=True,
                )
                nc.scalar.copy(gxn0[:, sl], ps[:, 0:C])

            # ---- time loop ----
            for t in range(n_steps):
                for l in range(3):
                    h = h_sb[l]
                    hprev = x0 if l == 0 else h_sb[l - 1]
                    wi = wih_sb[l]
                    wh = whh_sb[l]
                    npair = nchunk // 2
                    rows = H + VOCAB if l == 2 else H

                    t1 = wpool.tile([H + VOCAB, bs], BF16, tag="t1")
                    nbuf = wpool.tile([H, bs], BF16, tag="n")
                    dbuf = wpool.tile([H, bs], BF16, tag="d")
                    ebuf = wpool.tile([H, bs], BF16, tag="e")
                    rzb = rz2x if l == 2 else rzpool.tile([H, 2 * bs], BF16, tag="rz")
                    if l == 0:
                        t2 = wpool.tile([H, bs], BF16, tag="t2")

                    def gates(p):
                        X = xa if p % 2 == 0 else xb
                        for ci, c in enumerate((2 * p, 2 * p + 1)):
                            sl = slice(c * C, (c + 1) * C)
                            xh = slice(ci * C, (ci + 1) * C)
                            grz = ppool.tile([H, 2 * C], F32, tag="grz")
                            nc.tensor.matmul(
                                grz[:, 0:C], wh[:, 0:H], h[:, sl],
                                start=True, stop=False,
                            )
                            nc.tensor.matmul(
                                grz[:, 0:C], wi[:, 0:H], hprev[:, sl],
                                start=False, stop=True,
                            )
                            nc.tensor.matmul(
                                grz[:, C : 2 * C], wh[:, H : 2 * H], h[:, sl],
                                start=True, stop=False,
                            )
                            nc.tensor.matmul(
                                grz[:, C : 2 * C], wi[:, H : 2 * H], hprev[:, sl],
                                start=False, stop=True,
                            )
                            if l == 2:
                                nc.tensor.matmul(
                                    X[:, xh], wnout_sb[:, :], h[:, sl],
                                    start=True, stop=True,
                                )
                            else:
                                nc.tensor.matmul(
                                    X[0:H, xh], wh[:, 2 * H : 3 * H], h[:, sl],
                                    start=True, stop=True,
                                )
                            nc.scalar.activation(
                                rzb[0:H, c * 2 * C : (c + 1) * 2 * C],
                                grz[:, :], AF.Sigmoid,
                            )

                    def t1_pair(p):
                        # pair-granular t1 = r * ghn (layer 2: logits ride
                        # rows 100:120 against the constant-ones block)
                        X = xa if p % 2 == 0 else xb
                        c0 = 2 * p
                        psl = slice(c0 * C, (c0 + 2) * C)
                        r2 = rzb[0:rows, :].rearrange("p (a b) -> p a b", b=2 * C)[
                            :, c0 : c0 + 2, 0:C
                        ]
                        t1v = t1[0:rows, psl].rearrange("p (a b) -> p a b", b=C)
                        x2 = X[0:rows, :].rearrange("p (a b) -> p a b", b=C)
                        nc.vector.tensor_mul(t1v, r2, x2)

                    def ngate(p):
                        X = xa if p % 2 == 0 else xb
                        c0 = 2 * p
                        psl = slice(c0 * C, (c0 + 2) * C)
                        if l == 0:
                            nc.vector.tensor_add(
                                t2[:, psl], t1[0:H, psl], gxn0[:, psl]
                            )
                            nc.scalar.activation(
                                nbuf[:, psl], t2[:, psl], AF.Tanh
                            )
                        else:
                            for ci, c in enumerate((c0, c0 + 1)):
                                sl = slice(c * C, (c + 1) * C)
                                xh = slice(ci * C, (ci + 1) * C)
                                nc.tensor.matmul(
                                    X[0:H, xh], wi[:, 2 * H : 3 * H], hprev[:, sl],
                                    start=True, stop=False,
                                )
                                nc.tensor.matmul(
                                    X[0:H, xh], ident_sb[:, :], t1[0:H, sl],
                                    start=False, stop=True,
                                )
                            nc.scalar.activation(
                                nbuf[:, psl], X[0:H, :], AF.Tanh
                            )

                    # software-pipelined by one pair so the PE never waits
                    # on the sigma -> t1 chain of the pair it just fed
                    gates(0)
                    for p in range(npair):
                        if p + 1 < npair:
                            gates(p + 1)
                        t1_pair(p)
                        ngate(p)

                    # h' = n + z*(h - n), in half-batch pieces; the second
                    # half's subtract runs on the otherwise-idle GPSIMD
                    for hf in range(2):
                        hs = slice(hf * nhalf, (hf + 1) * nhalf)
                        nh = nhalf // C
                        if hf == 0:
                            nc.vector.tensor_sub(dbuf[:, hs], h[0:H, hs], nbuf[:, hs])
                        else:
                            nc.gpsimd.tensor_sub(dbuf[:, hs], h[0:H, hs], nbuf[:, hs])
                        z3 = rzb[0:H, :].rearrange("p (a b) -> p a b", b=2 * C)[
                            :, hf * nh : (hf + 1) * nh, C : 2 * C
                        ]
                        d3 = dbuf[:, hs].rearrange("p (a b) -> p a b", b=C)
                        e3 = ebuf[:, hs].rearrange("p (a b) -> p a b", b=C)
                        nc.vector.tensor_mul(e3, z3, d3)
                        nc.vector.tensor_add(h[0:H, hs], nbuf[:, hs], ebuf[:, hs])

                    if l == 2 and t > 0:
                        nc.sync.dma_start(out[t - 1, :, :], t1[96 : H + VOCAB, :])

            # ---- epilogue: logits for the last step ----
            lgbuf = wpool.tile([VP, bs], BF16, tag="t2")
            for p in range(nchunk // 2):
                X = xa if p % 2 == 0 else xb
                for ci, c in enumerate((2 * p, 2 * p + 1)):
                    sl = slice(c * C, (c + 1) * C)
                    xh = slice(ci * C, (ci + 1) * C)
                    nc.tensor.matmul(
                        X[:, xh], wnout_sb[:, :], h_sb[2][:, sl],
                        start=True, stop=True,
                    )
                psl = slice(2 * p * C, (2 * p + 2) * C)
                lgv = lgbuf[:, psl].rearrange("p (a b) -> p a b", b=C)
                xv = X[96 : H + VOCAB, :].rearrange("p (a b) -> p a b", b=C)
                nc.vector.tensor_copy(lgv, xv)
            nc.sync.dma_start(out[n_steps - 1, :, :], lgbuf[:, :])

    nc.finalize()
    return nc


def _prep_weights(
    w_proj,
    b_proj,
    wih0,
    whh0,
    bih0,
    bhh0,
    wih1,
    whh1,
    bih1,
    bhh1,
    wih2,
    whh2,
    bih2,
    bhh2,
    w_out,
    b_out,
):
    """Host-side packing: transpose weights, fold b_proj into layer-0 input
    bias, append bias rows, pack w_out into the layer-2 n-gate stationary."""
    bf16 = ml_dtypes.bfloat16
    f32 = np.float32

    def stat(w, b):
        # [out, in] weight + [out] bias -> [in+1, out] stationary
        return np.concatenate([w.T, b[None, :]], axis=0).astype(bf16)

    bih0_eff = (bih0 + wih0 @ b_proj).astype(f32)
    wihT = [stat(wih0, bih0_eff), stat(wih1, bih1), stat(wih2, bih2)]
    whhT = [stat(whh0, bhh0), stat(whh1, bhh1), stat(whh2, bhh2)]
    wout_stat = stat(w_out, b_out)  # [101, 20]
    wnout = np.concatenate([whhT[2][:, 2 * H : 3 * H], wout_stat], axis=1)
    return {
        "wproj": w_proj.T.astype(bf16),
        "wih0": wihT[0],
        "wih1": wihT[1],
        "wih2": wihT[2],
        "whh0": whhT[0],
        "whh1": whhT[1],
        "whh2": whhT[2],
        "wnout": np.ascontiguousarray(wnout).astype(bf16),
        "ident": np.eye(H, dtype=bf16),
    }


_NC_CACHE = {}


def _get_nc():
    if "nc" not in _NC_CACHE:
        _NC_CACHE["nc"] = build_nc()
    return _NC_CACHE["nc"]


def kernel(
    encoder_hidden,
    w_proj,
    b_proj,
    wih0,
    whh0,
    bih0,
    bhh0,
    wih1,
    whh1,
    bih1,
    bhh1,
    wih2,
    whh2,
    bih2,
    bhh2,
    w_out,
    b_out,
    _trace=False,
):
    f32 = np.float32
    encoder_hidden = np.asarray(encoder_hidden, f32)
    args = [
        np.asarray(a, f32)
        for a in (
            w_proj,
            b_proj,
            wih0,
            whh0,
            bih0,
            bhh0,
            wih1,
            whh1,
            bih1,
            bhh1,
            wih2,
            whh2,
            bih2,
            bhh2,
            w_out,
            b_out,
        )
    ]
    weights = _prep_weights(*args)

    ehT = np.ascontiguousarray(encoder_hidden.T).astype(ml_dtypes.bfloat16)
    in_maps = []
    for i in range(N_CORES):
        m = dict(weights)
        m["ehT"] = np.ascontiguousarray(ehT[:, i * BS : (i + 1) * BS])
        in_maps.append(m)

    nc = _get_nc()
    res = run_bass_kernel_spmd(
        nc, in_maps, core_ids=list(range(N_CORES)), trace=_trace
    )
    outs = [
        np.asarray(res.results[i]["out"], f32).transpose(2, 0, 1)[:, :, 4:]
        for i in range(N_CORES)
    ]
    full = np.concatenate(outs, axis=0)
    if _trace:
        kernel.last_exec_time_ns = res.exec_time_ns
        kernel.last_results = res
    return full


# revision 12
# speedup vs baseline: 1.1976x; 1.1976x over previous
"""Trainium2 Bass kernel for a 3-layer GRU decoder (DecoderRNN).

Math (per timestep, identical input x0 each step):
    x0 = encoder_hidden @ w_proj.T + b_proj
    3 stacked GRU layers (PyTorch gate order r,z,n), then logits = h2 @ w_out.T + b_out

Device mapping (per core, batch shard BS=4096):
  - Layout: features on SBUF partitions, batch on the free dim.
  - Weights pre-transposed/packed on host; biases ride in an extra
    contraction row against a constant 1.0 row held in each state tile.
  - Gate pre-activations accumulate in PSUM; sigmoid(r,z) is one merged
    ACT op per chunk; the n-gate bank is recycled: ghn -> (t1 read) ->
    gxn + identity@t1 -> tanh, so the gxn+r*ghn add runs on the PE.
  - w_out is packed into the layer-2 whh_n stationary; logits(t-1) ride
    the t1 multiply (rows 100:120 against a constant-ones block) straight
    into SBUF, then one DMA per step.
  - Output is [T, 24, BS] bf16 per core (rows 4:24 are logits); host
    transposes/casts back to [B, T, VOCAB] fp32.
"""

import numpy as np
import ml_dtypes

import concourse.bass as bass
import concourse.mybir as mybir
from concourse import bacc
from concourse.tile import TileContext
from concourse.bass_utils import run_bass_kernel_spmd

N_CORES = 8
B = 32768
BS = B // N_CORES  # 4096
LATENT = 128
H = 100
VOCAB = 20
VP = VOCAB + 4  # logits rows padded to a 32-aligned partition window (96:120)
T = 21
C = 512  # batch chunk = one PSUM bank of fp32

BF16 = mybir.dt.bfloat16
F32 = mybir.dt.float32
AF = mybir.ActivationFunctionType
OP = mybir.AluOpType


def build_nc(bs=BS, n_steps=T):
    nchunk = bs // C
    nhalf = bs // 2
    nc = bacc.Bacc("TRN2", target_bir_lowering=False)

    ehT = nc.declare_dram_parameter("ehT", [LATENT, bs], BF16, isOutput=False)
    wproj = nc.declare_dram_parameter("wproj", [LATENT, H], BF16, isOutput=False)
    wih = [
        nc.declare_dram_parameter(f"wih{l}", [H + 1, 3 * H], BF16, isOutput=False)
        for l in range(3)
    ]
    whh = [
        nc.declare_dram_parameter(f"whh{l}", [H + 1, 3 * H], BF16, isOutput=False)
        for l in range(3)
    ]
    # layer-2 n-gate stationary with w_out packed in cols 100:120
    wnout = nc.declare_dram_parameter("wnout", [H + 1, H + VOCAB], BF16, isOutput=False)
    ident = nc.declare_dram_parameter("ident", [H, H], BF16, isOutput=False)
    out = nc.declare_dram_parameter("out", [n_steps, VP, bs], BF16, isOutput=True)

    with TileContext(nc) as tc:
        with (
            tc.tile_pool(name="const", bufs=1) as cpool,
            tc.tile_pool(name="state", bufs=1) as spool,
            tc.tile_pool(name="rz", bufs=2) as rzpool,
            tc.tile_pool(name="work", bufs=2) as wpool,
            tc.tile_pool(name="psum", bufs=2, space="PSUM") as ppool,
            tc.tile_pool(name="psumx", bufs=1, space="PSUM") as xpool,
        ):
            # ---- load weights ----
            eh_sb = cpool.tile([LATENT, bs], BF16, tag="eh")
            nc.sync.dma_start(eh_sb[:, :], ehT[:, :])
            wproj_sb = cpool.tile([LATENT, H], BF16, tag="wproj")
            nc.sync.dma_start(wproj_sb[:, :], wproj[:, :])
            wih_sb = []
            whh_sb = []
            for l in range(3):
                wi = cpool.tile([H + 1, 3 * H], BF16, tag=f"wih{l}")
                nc.sync.dma_start(wi[:, :], wih[l][:, :])
                wih_sb.append(wi)
                wh = cpool.tile([H + 1, 3 * H], BF16, tag=f"whh{l}")
                nc.sync.dma_start(wh[:, :], whh[l][:, :])
                whh_sb.append(wh)
            wnout_sb = cpool.tile([H + 1, H + VOCAB], BF16, tag="wnout")
            nc.sync.dma_start(wnout_sb[:, :], wnout[:, :])
            ident_sb = cpool.tile([H, H], BF16, tag="ident")
            nc.sync.dma_start(ident_sb[:, :], ident[:, :])

            # ---- state tiles ----
            # ones-row writes start at partition 96 (HW requires 32-aligned
            # partition bases), then zero back rows 96:100
            h_sb = []
            for l in range(3):
                h = spool.tile([H + 1, bs], BF16, tag=f"h{l}")
                nc.gpsimd.memset(h[:, :], 0.0)
                nc.gpsimd.memset(h[96 : H + 1, :], 1.0)
                nc.gpsimd.memset(h[96:H, :], 0.0)
                h_sb.append(h)
            x0 = spool.tile([H + 1, bs], BF16, tag="x0")
            nc.gpsimd.memset(x0[96 : H + 1, :], 1.0)
            nc.gpsimd.memset(x0[96:H, :], 0.0)
            gxn0 = spool.tile([H, bs], BF16, tag="gxn0")
            # layer-2 (r|z) buffer with a constant-ones block on rows
            # 100:120 (rows 96:100 are rewritten by sigma each step)
            rz2x = spool.tile([H + VOCAB, 2 * bs], BF16, tag="rz2x")
            nc.gpsimd.memset(rz2x[96 : H + VOCAB, :], 1.0)
            # static PSUM pair tiles for the n-gate bank recycle
            xa = xpool.tile([H + VOCAB, 2 * C], F32, tag="xa")
            xb = xpool.tile([H + VOCAB, 2 * C], F32, tag="xb")

            # ---- prologue: x0 = wproj.T @ ehT ; gxn0 = wih0_n.T @ x0 ----
            for c in range(nchunk):
                sl = slice(c * C, (c + 1) * C)
                ps = ppool.tile([H, 2 * C], F32, tag="grz")
                nc.tensor.matmul(
                    ps[:, 0:C], wproj_sb[:, :], eh_sb[:, sl], start=True, stop=True
                )
                nc.scalar.copy(x0[0:H, sl], ps[:, 0:C])
            for c in range(nchunk):
                sl = slice(c * C, (c + 1) * C)
                ps = ppool.tile([H, 2 * C], F32, tag="grz")
                nc.tensor.matmul(
                    ps[:, 0:C],
                    wih_sb[0][:, 2 * H : 3 * H],
                    x0[:, sl],
                    start=True,
                    stop=True,
                )
                nc.scalar.copy(gxn0[:, sl], ps[:, 0:C])

            # ---- time loop ----
            for t in range(n_steps):
                for l in range(3):
                    h = h_sb[l]
                    hprev = x0 if l == 0 else h_sb[l - 1]
                    wi = wih_sb[l]
                    wh = whh_sb[l]
                    npair = nchunk // 2
                    rows = H + VOCAB if l == 2 else H

                    t1 = wpool.tile([H + VOCAB, bs], BF16, tag="t1")
                    nbuf = wpool.tile([H, bs], BF16, tag="n")
                    dbuf = wpool.tile([H, bs], BF16, tag="d")
                    ebuf = wpool.tile([H, bs], BF16, tag="e")
                    rzb = rz2x if l == 2 else rzpool.tile([H, 2 * bs], BF16, tag="rz")
                    if l == 0:
                        t2 = wpool.tile([H, bs], BF16, tag="t2")

                    def gates(p):
                        X = xa if p % 2 == 0 else xb
                        for ci, c in enumerate((2 * p, 2 * p + 1)):
                            sl = slice(c * C, (c + 1) * C)
                            xh = slice(ci * C, (ci + 1) * C)
                            grz = ppool.tile([H, 2 * C], F32, tag="grz")
                            nc.tensor.matmul(
                                grz[:, 0:C], wh[:, 0:H], h[:, sl],
                                start=True, stop=False,
                            )
                            nc.tensor.matmul(
                                grz[:, 0:C], wi[:, 0:H], hprev[:, sl],
                                start=False, stop=True,
                            )
                            nc.tensor.matmul(
                                grz[:, C : 2 * C], wh[:, H : 2 * H], h[:, sl],
                                start=True, stop=False,
                            )
                            nc.tensor.matmul(
                                grz[:, C : 2 * C], wi[:, H : 2 * H], hprev[:, sl],
                                start=False, stop=True,
                            )
                            if l == 2:
                                nc.tensor.matmul(
                                    X[:, xh], wnout_sb[:, :], h[:, sl],
                                    start=True, stop=True,
                                )
                            else:
                                nc.tensor.matmul(
                                    X[0:H, xh], wh[:, 2 * H : 3 * H], h[:, sl],
                                    start=True, stop=True,
                                )
                            nc.scalar.activation(
                                rzb[0:H, c * 2 * C : (c + 1) * 2 * C],
                                grz[:, :], AF.Sigmoid,
                            )

                    def t1_pair(p):
                        # pair-granular t1 = r * ghn (layer 2: logits ride
                        # rows 100:120 against the constant-ones block)
                        X = xa if p % 2 == 0 else xb
                        c0 = 2 * p
                        psl = slice(c0 * C, (c0 + 2) * C)
                        r2 = rzb[0:rows, :].rearrange("p (a b) -> p a b", b=2 * C)[
                            :, c0 : c0 + 2, 0:C
                        ]
                        t1v = t1[0:rows, psl].rearrange("p (a b) -> p a b", b=C)
                        x2 = X[0:rows, :].rearrange("p (a b) -> p a b", b=C)
                        nc.vector.tensor_mul(t1v, r2, x2)

                    def ngate(p):
                        X = xa if p % 2 == 0 else xb
                        c0 = 2 * p
                        psl = slice(c0 * C, (c0 + 2) * C)
                        if l == 0:
                            nc.vector.tensor_add(
                                t2[:, psl], t1[0:H, psl], gxn0[:, psl]
                            )
                            nc.scalar.activation(
                                nbuf[:, psl], t2[:, psl], AF.Tanh
                            )
                        else:
                            for ci, c in enumerate((c0, c0 + 1)):
                                sl = slice(c * C, (c + 1) * C)
                                xh = slice(ci * C, (ci + 1) * C)
                                nc.tensor.matmul(
                                    X[0:H, xh], wi[:, 2 * H : 3 * H], hprev[:, sl],
                                    start=True, stop=False,
                                )
                                nc.tensor.matmul(
                                    X[0:H, xh], ident_sb[:, :], t1[0:H, sl],
                                    start=False, stop=True,
                                )
                            nc.scalar.activation(
                                nbuf[:, psl], X[0:H, :], AF.Tanh
                            )

                    # software-pipelined by one pair so the PE never waits
                    # on the sigma -> t1 chain of the pair it just fed
                    gates(0)
                    for p in range(npair):
                        if p + 1 < npair:
                            gates(p + 1)
                        t1_pair(p)
                        ngate(p)

                    # h' = n + z*(h - n), in half-batch pieces for pipelining
                    for hf in range(2):
                        hs = slice(hf * nhalf, (hf + 1) * nhalf)
                        nh = nhalf // C
                        nc.vector.tensor_sub(dbuf[:, hs], h[0:H, hs], nbuf[:, hs])
                        z3 = rzb[0:H, :].rearrange("p (a b) -> p a b", b=2 * C)[
                            :, hf * nh : (hf + 1) * nh, C : 2 * C
                        ]
                        d3 = dbuf[:, hs].rearrange("p (a b) -> p a b", b=C)
                        e3 = ebuf[:, hs].rearrange("p (a b) -> p a b", b=C)
                        nc.vector.tensor_mul(e3, z3, d3)
                        nc.vector.tensor_add(h[0:H, hs], nbuf[:, hs], ebuf[:, hs])

                    if l == 2 and t > 0:
                        nc.sync.dma_start(out[t - 1, :, :], t1[96 : H + VOCAB, :])

            # ---- epilogue: logits for the last step ----
            lgbuf = wpool.tile([VP, bs], BF16, tag="t2")
            for p in range(nchunk // 2):
                X = xa if p % 2 == 0 else xb
                for ci, c in enumerate((2 * p, 2 * p + 1)):
                    sl = slice(c * C, (c + 1) * C)
                    xh = slice(ci * C, (ci + 1) * C)
                    nc.tensor.matmul(
                        X[:, xh], wnout_sb[:, :], h_sb[2][:, sl],
                        start=True, stop=True,
                    )
                psl = slice(2 * p * C, (2 * p + 2) * C)
                lgv = lgbuf[:, psl].rearrange("p (a b) -> p a b", b=C)
                xv = X[96 : H + VOCAB, :].rearrange("p (a b) -> p a b", b=C)
                nc.vector.tensor_copy(lgv, xv)
            nc.sync.dma_start(out[n_steps - 1, :, :], lgbuf[:, :])

    nc.finalize()
    return nc


def _prep_weights(
    w_proj,
    b_proj,
    wih0,
    whh0,
    bih0,
    bhh0,
    wih1,
    whh1,
    bih1,
    bhh1,
    wih2,
    whh2,
    bih2,
    bhh2,
    w_out,
    b_out,
):
    """Host-side packing: transpose weights, fold b_proj into layer-0 input
    bias, append bias rows, pack w_out into the layer-2 n-gate stationary."""
    bf16 = ml_dtypes.bfloat16
    f32 = np.float32

    def stat(w, b):
        # [out, in] weight + [out] bias -> [in+1, out] stationary
        return np.concatenate([w.T, b[None, :]], axis=0).astype(bf16)

    bih0_eff = (bih0 + wih0 @ b_proj).astype(f32)
    wihT = [stat(wih0, bih0_eff), stat(wih1, bih1), stat(wih2, bih2)]
    whhT = [stat(whh0, bhh0), stat(whh1, bhh1), stat(whh2, bhh2)]
    wout_stat = stat(w_out, b_out)  # [101, 20]
    wnout = np.concatenate([whhT[2][:, 2 * H : 3 * H], wout_stat], axis=1)
    return {
        "wproj": w_proj.T.astype(bf16),
        "wih0": wihT[0],
        "wih1": wihT[1],
        "wih2": wihT[2],
        "whh0": whhT[0],
        "whh1": whhT[1],
        "whh2": whhT[2],
        "wnout": np.ascontiguousarray(wnout).astype(bf16),
        "ident": np.eye(H, dtype=bf16),
    }


_NC_CACHE = {}


def _get_nc():
    if "nc" not in _NC_CACHE:
        _NC_CACHE["nc"] = build_nc()
    return _NC_CACHE["nc"]


def kernel(
    encoder_hidden,
    w_proj,
    b_proj,
    wih0,
    whh0,
    bih0,
    bhh0,
    wih1,
    whh1,
    bih1,
    bhh1,
    wih2,
    whh2,
    bih2,
    bhh2,
    w_out,
    b_out,
    _trace=False,
):
    f32 = np.float32
    encoder_hidden = np.asarray(encoder_hidden, f32)
    args = [
        np.asarray(a, f32)
        for a in (
            w_proj,
            b_proj,
            wih0,
            whh0,
            bih0,
            bhh0,
            wih1,
            whh1,
            bih1,
            bhh1,
            wih2,
            whh2,
            bih2,
            bhh2,
            w_out,
            b_out,
        )
    ]
    weights = _prep_weights(*args)

    ehT = np.ascontiguousarray(encoder_hidden.T).astype(ml_dtypes.bfloat16)
    in_maps = []
    for i in range(N_CORES):
        m = dict(weights)
        m["ehT"] = np.ascontiguousarray(ehT[:, i * BS : (i + 1) * BS])
        in_maps.append(m)

    nc = _get_nc()
    res = run_bass_kernel_spmd(
        nc, in_maps, core_ids=list(range(N_CORES)), trace=_trace
    )
    outs = [
        np.asarray(res.results[i]["out"], f32).transpose(2, 0, 1)[:, :, 4:]
        for i in range(N_CORES)
    ]
    full = np.concatenate(outs, axis=0)
    if _trace:
        kernel.last_exec_time_ns = res.exec_time_ns
        kernel.last_results = res
    return full
